# revision 7
# baseline (speedup 1.0000x reference)
"""AdaptiveCenterLoss on 8 TRN2 NeuronCores.

loss = mean_i ||features[i] - centers[labels[i]]||^2
     with B=131072, D=256, C=1000.

Strategy (data-parallel, memory-bound):
  - host-side, sort rows by label and pack them into one-label blocks of
    16/8/4/2/1 rows (binary decomposition of each class count, then
    leftover blocks of each size are split in two and demoted until each
    size's block count is an exact multiple of 8 cores x 128 partitions)
    -> padding is <0.01%.
  - the bulk (16-row blocks) ships as fp8e4 and is cast to bf16 during
    the SWDGE DMA (halves the dominant HBM traffic again vs bf16; the
    tolerance is 2e-2 and fp8 features cost ~4e-4).  The small-block
    tail and the per-block centers stay bf16 on HWDGE.
  - per-block center rows are materialized host-side into a dense
    [P, T, D] tensor per core -> no indirect DMA at all.
  - per tile: DVE subtract in bf16 (2x_1P packed mode; the center
    broadcast rides a stride-0 middle dim, innermost stays step-1).
    The square+sum is spread across THREE engines: ACT Square
    activation, DVE mult-accumulate, and TensorE (PSUM-accumulated
    Gram matmuls of the diff tile whose diagonal -- extracted with one
    tiny identity mult-accum -- is exactly sum(diff^2) per partition).
  - each core outputs per-tile partial sums (one bank per engine so
    every SBUF tile is single-writer); host sums and divides by B.
"""

import numpy as np
import ml_dtypes

import concourse.bacc as bacc
import concourse.bass as bass
import concourse.mybir as mybir
import concourse.tile as tile
from concourse.bass_utils import run_bass_kernel_spmd

B, D, C = 131072, 256, 1000
N_CORES = 8
P = 128
GROUP = N_CORES * P
SIZES = (16, 8, 4, 2, 1)
BF16 = ml_dtypes.bfloat16
FP8 = ml_dtypes.float8_e4m3

_nc_cache = {}


def _plan(slots_list):
    """Processing order and square-engine assignment for each tile."""
    T = len(slots_list)
    big = [t for t in range(T) if slots_list[t] == 16]
    small = [t for t in range(T) if slots_list[t] != 16]
    small_sorted = sorted(small, key=lambda t: slots_list[t])
    tail = [small_sorted[0]] if small_sorted else []
    head = small_sorted[1:]
    order = head + big + tail

    sq_engine = {t: "act" for t in range(T)}
    pattern = ["act", "te", "te", "te", "act", "stt", "act"]
    for i, t in enumerate(big):
        sq_engine[t] = pattern[i % len(pattern)]
    return order, sq_engine


def _build(slots_list):
    key = tuple(slots_list)
    if key in _nc_cache:
        return _nc_cache[key]
    T = len(slots_list)
    order, sq_engine = _plan(slots_list)
    n16 = sum(1 for s in slots_list if s == 16)
    rows16 = P * 16 * n16
    rows_sm = P * sum(s for s in slots_list if s != 16)

    nc = bacc.Bacc()
    feats8 = nc.declare_dram_parameter(
        "feats8", [rows16, D], mybir.dt.float8e4, isOutput=False
    )
    feats_sm = nc.declare_dram_parameter(
        "feats_sm", [rows_sm, D], mybir.dt.bfloat16, isOutput=False
    )
    cents = nc.declare_dram_parameter(
        "cents", [P, T * D], mybir.dt.bfloat16, isOutput=False
    )
    ident = nc.declare_dram_parameter(
        "ident", [P, P], mybir.dt.bfloat16, isOutput=False
    )
    out_a = nc.declare_dram_parameter("out_a", [P, T], mybir.dt.float32, isOutput=True)
    out_d = nc.declare_dram_parameter("out_d", [P, T], mybir.dt.float32, isOutput=True)

    # canonical row offsets: 16-region rows live in feats8, the rest in
    # feats_sm (both in canonical tile order)
    rowbase = {}
    rb8 = 0
    rbs = 0
    for t, s in enumerate(slots_list):
        if s == 16:
            rowbase[t] = rb8
            rb8 += P * s
        else:
            rowbase[t] = rbs
            rbs += P * s

    with tile.TileContext(nc) as tc:
        with (
            tc.tile_pool(name="c", bufs=1) as c_pool,
            tc.tile_pool(name="f", bufs=8) as f_pool,
            tc.tile_pool(name="sq", bufs=2) as sq_pool,
            tc.tile_pool(name="acc", bufs=1) as acc_pool,
            tc.tile_pool(name="ps", bufs=2, space=bass.MemorySpace.PSUM) as ps_pool,
        ):
            call = c_pool.tile([P, T * D], mybir.dt.bfloat16)
            nc.sync.dma_start(out=call[:], in_=cents[:])
            idt = c_pool.tile([P, P], mybir.dt.bfloat16)
            nc.sync.dma_start(out=idt[:], in_=ident[:])
            acc_a = acc_pool.tile([P, T], mybir.dt.float32, tag="aa")
            acc_d = acc_pool.tile([P, T], mybir.dt.float32, tag="ad")
            for t in order:
                slots = slots_list[t]
                f_t = f_pool.tile([P, slots * D], mybir.dt.bfloat16, tag="f")
                if slots == 16:
                    nc.gpsimd.dma_start(
                        out=f_t[:].rearrange("p (s d) -> p s d", s=slots),
                        in_=feats8[rowbase[t] : rowbase[t] + P * slots, :].rearrange(
                            "(p s) d -> p s d", p=P
                        ),
                    )
                else:
                    nc.sync.dma_start(
                        out=f_t[:].rearrange("p (s d) -> p s d", s=slots),
                        in_=feats_sm[rowbase[t] : rowbase[t] + P * slots, :].rearrange(
                            "(p s) d -> p s d", p=P
                        ),
                    )
                c_b = (
                    call[:, t * D : (t + 1) * D]
                    .rearrange("p (s d) -> p s d", s=1)
                    .to_broadcast([P, slots, D])
                )
                nc.vector.tensor_tensor(
                    out=f_t[:].rearrange("p (s d) -> p s d", s=slots),
                    in0=f_t[:].rearrange("p (s d) -> p s d", s=slots),
                    in1=c_b,
                    op=mybir.AluOpType.subtract,
                )
                eng = sq_engine[t]
                if eng == "act":
                    nc.scalar.activation(
                        out=f_t[:],
                        in_=f_t[:],
                        func=mybir.ActivationFunctionType.Square,
                        accum_out=acc_a[:, t : t + 1],
                    )
                elif eng == "stt":
                    sq_t = sq_pool.tile([P, slots * D], mybir.dt.bfloat16, tag="sq")
                    nc.vector.scalar_tensor_tensor(
                        out=sq_t[:],
                        in0=f_t[:],
                        scalar=0.0,
                        in1=f_t[:],
                        op0=mybir.AluOpType.bypass,
                        op1=mybir.AluOpType.mult,
                        accum_out=acc_d[:, t : t + 1],
                    )
                elif eng == "te":
                    # Gram trick: accumulate X_c^T X_c over the 32 column
                    # chunks of the diff tile; diag(PSUM)[n] = sum_p x[p,n]^2
                    # so identity-mask + row-sum-accumulate = sum(diff^2).
                    ps = ps_pool.tile([P, P], mybir.dt.float32, tag="ps")
                    nch = (slots * D) // P
                    for i in range(nch):
                        nc.tensor.matmul(
                            ps[:],
                            f_t[:, i * P : (i + 1) * P],
                            f_t[:, i * P : (i + 1) * P],
                            start=(i == 0),
                            stop=(i == nch - 1),
                        )
                    scr = sq_pool.tile([P, P], mybir.dt.float32, tag="scr")
                    nc.vector.scalar_tensor_tensor(
                        out=scr[:],
                        in0=ps[:],
                        scalar=0.0,
                        in1=idt[:],
                        op0=mybir.AluOpType.bypass,
                        op1=mybir.AluOpType.mult,
                        accum_out=acc_d[:, t : t + 1],
                    )
            nc.sync.dma_start(out=out_a[:], in_=acc_a[:])
            nc.sync.dma_start(out=out_d[:], in_=acc_d[:])
    nc.finalize()
    _nc_cache[key] = nc
    return nc


def _pack(labels):
    """Cascade packing: per-class block counts per size, tile counts, and
    the class of every block position in the (size, core, tile, partition)
    grid."""
    counts = np.bincount(labels, minlength=C).astype(np.int64)
    nblk = {16: counts // 16}
    rem = counts % 16
    for s in (8, 4, 2, 1):
        nblk[s] = (rem // s) % 2
    for s in (16, 8, 4, 2):
        Ns = int(nblk[s].sum())
        Ls = Ns % GROUP
        if Ls:
            cum = np.cumsum(nblk[s])
            dem = np.clip(cum - (Ns - Ls), 0, nblk[s])
            nblk[s] = nblk[s] - dem
            nblk[s // 2] = nblk[s // 2] + 2 * dem
    pad1 = (-int(nblk[1].sum())) % GROUP

    tiles_per_size = {s: int(nblk[s].sum()) // GROUP for s in SIZES}
    tiles_per_size[1] = (int(nblk[1].sum()) + pad1) // GROUP
    blist = {}
    for s in SIZES:
        bl = np.repeat(np.arange(C, dtype=np.int32), nblk[s])
        if s == 1 and pad1:
            bl = np.concatenate([bl, np.zeros(pad1, dtype=np.int32)])
        blist[s] = bl
    return counts, nblk, tiles_per_size, blist, pad1


def _prepare(features, centers, labels):
    features = np.asarray(features)
    centers_f = np.ascontiguousarray(np.asarray(centers), dtype=np.float32)
    centers16 = centers_f.astype(BF16)
    labels = np.asarray(labels).astype(np.int64)

    counts, nblk, tiles_per_size, blist, pad1 = _pack(labels)

    slots_list = []
    for s in SIZES:
        slots_list += [s] * tiles_per_size[s]
    T = len(slots_list)
    rows_core = P * sum(slots_list)
    rows16 = P * 16 * tiles_per_size[16]

    base_off = {}
    off = 0
    for s in SIZES:
        base_off[s] = off
        off += tiles_per_size[s] * P * s
    assert off == rows_core

    order = np.argsort(labels, kind="stable")
    labels_sorted = labels[order]
    class_row_start = np.concatenate(([0], np.cumsum(counts)[:-1]))
    rank = np.arange(B, dtype=np.int64) - class_row_start[labels_sorted]

    dst = np.empty(B, dtype=np.int64)
    lo = np.zeros(C, dtype=np.int64)
    for s in SIZES:
        ns = nblk[s]
        hi = lo + s * ns
        m = (rank >= lo[labels_sorted]) & (rank < hi[labels_sorted])
        if m.any():
            j = labels_sorted[m]
            r = rank[m] - lo[j]
            start_s = np.concatenate(([0], np.cumsum(ns)[:-1]))
            bidx = start_s[j] + r // s
            JP = tiles_per_size[s] * P
            core = bidx // JP
            rem_b = bidx % JP
            dst[m] = core * rows_core + base_off[s] + rem_b * s + r % s
        lo = hi

    fpad = np.empty((N_CORES * rows_core, D), dtype=np.float32)
    fpad[dst] = features.astype(np.float32)[order]
    if pad1:
        JP = tiles_per_size[1] * P
        bidx = np.arange(len(blist[1]) - pad1, len(blist[1]), dtype=np.int64)
        core = bidx // JP
        rem_b = bidx % JP
        rows = core * rows_core + base_off[1] + rem_b
        fpad[rows] = centers16[0].astype(np.float32)

    ident = np.eye(P, dtype=BF16)
    maps = []
    for k in range(N_CORES):
        cw = np.empty((P, T, D), dtype=BF16)
        t0 = 0
        for s in SIZES:
            Js = tiles_per_size[s]
            if Js == 0:
                continue
            cls = blist[s][k * Js * P : (k + 1) * Js * P].reshape(Js, P)
            cw[:, t0 : t0 + Js, :] = centers16[cls].transpose(1, 0, 2)
            t0 += Js
        fs = fpad[k * rows_core : (k + 1) * rows_core]
        maps.append(
            {
                "feats8": np.ascontiguousarray(fs[:rows16]).astype(FP8),
                "feats_sm": np.ascontiguousarray(fs[rows16:]).astype(BF16),
                "cents": np.ascontiguousarray(cw.reshape(P, T * D)),
                "ident": ident,
            }
        )
    return maps, slots_list


def run(features, centers, labels, trace=False):
    maps, slots_list = _prepare(features, centers, labels)
    nc = _build(slots_list)
    _, sq_engine = _plan(slots_list)
    res = run_bass_kernel_spmd(
        nc, maps, core_ids=list(range(N_CORES)), trace=trace
    )
    act_cols = [t for t, e in sq_engine.items() if e == "act"]
    dve_cols = [t for t, e in sq_engine.items() if e != "act"]
    total = 0.0
    for r in res.results:
        total += float(np.asarray(r["out_a"])[:, act_cols].astype(np.float64).sum())
        total += float(np.asarray(r["out_d"])[:, dve_cols].astype(np.float64).sum())
    return np.float32(total / B), res


def kernel(features, centers, labels):
    last_err = None
    for _ in range(3):
        try:
            loss, _ = run(features, centers, labels)
            return loss
        except Exception as e:  # noqa: BLE001
            last_err = e
    raise last_err
